# revision 5
# baseline (speedup 1.0000x reference)
"""Trainium2 Bass kernel for nn_ComplexQuantumLayer (10-qubit circuit, batch 2048).

Math: the circuit after the RX AngleEmbedding is a fixed unitary U (depends only
on `weights`), and the embedded state is a Kronecker product
  psi0[b] = (-i)^popcount(j) * m[b, j],   m[b] = kron_q [cos(x_bq/2), sin(x_bq/2)].
Folding the phase into W = diag(phase) @ U^T gives  psi = m @ W  with m REAL.
Per sample the device does two real (1024,1024) matvecs (fp16 operands, fp32
PSUM), |psi|^2, and the ten PauliZ sums as one more small matmul against a
+/-1 mask matrix.

v3 design — 2D sharding (batch x out-amp) to halve per-core DMA:
  Cores are paired: pair c covers samples [512c, 512c+512); the even core
  computes out-amp chunks 0-3, the odd core chunks 4-7. Each core loads HALF
  of W (2.23MB) plus mt for its sample block (1MB) = 3.25MB, well under the
  ~330GB/s per-core DMA ceiling for the ~15us PE stream; the host adds the
  two partial Z sums of a pair (both cover disjoint out-amps).

  - The host sends m TRANSPOSED (amp-major): mt[p, k, b] = m[b, k*128+p].
    No on-device Kronecker tree or transposes; the PE runs only productive
    matmuls.
  - DMA rings alternate between the two hardware DGE queues (sync/scalar) in
    stream-consumption order, with mt and the first W chunk split into small
    granules so the matmul stream starts as early as possible:
      sync  : mt[0:2] | W0re[k0:4] | mt[4:6] | W0re[k4:8] | W1re | W2re | W3re
      scalar: mt[2:4] | W0im[k0:4] | mt[6:8] | W0im[k4:8]+mask | W1im | ...
  - Main stream per out-chunk j: 8 re matmuls -> ps_r, 8 im -> ps_i
    ([128,512] fp32 PSUM, one bank each). ACT squares both into fp16, DVE
    adds -> p[j]; one Z-mask matmul per chunk (lhsT [128,32] zero-padded,
    full rate) accumulates into zp [32,512]. Z[j] is issued one chunk late
    (between chunk j+1's re and im passes) so the PE never waits on the
    ACT/DVE probs chain; the LAST chunk's probs+Z are column-split so only
    a ~0.9us chain remains after the final im matmul.
  - ~20 PE warm-up matmuls on a memset dummy ramp the tensor clock during
    the initial DMA wait.
"""

import numpy as np

import concourse.bass as bass
import concourse.bacc as bacc
import concourse.mybir as mybir
from concourse.bass_utils import run_bass_kernel_spmd
from concourse.tile import TileContext

NQ = 10
DIM = 1 << NQ          # 1024
BATCH = 2048
NCORES = 8
NPAIR = NCORES // 2    # 4 sample blocks
BPC = BATCH // NPAIR   # 512 samples per core (shared by a pair)
P = 128                # partitions
KC = DIM // P          # in-amp chunks = 8
JC = 4                 # out-amp chunks per core (half of 8)

F32 = mybir.dt.float32
F16 = mybir.dt.float16
MUL = mybir.AluOpType.mult
ADD = mybir.AluOpType.add

LAST_RESULT = None  # BassKernelResults of the most recent run (for test harness)


# ----------------------------------------------------------------------------
# Host-side preprocessing: circuit unitary from weights (numpy, ~2s)
# ----------------------------------------------------------------------------

def _build_circuit_matrix(weights: np.ndarray, dtype=np.complex128) -> np.ndarray:
    """M = U^T: the reference circuit (post-embedding) applied to identity rows."""
    w = weights.astype(np.float64)
    state = np.eye(DIM, dtype=dtype)

    def apply_1q(state, g, q):
        s = state.reshape(DIM, 1 << q, 2, -1)
        s0 = s[:, :, 0, :].copy()
        s1 = s[:, :, 1, :].copy()
        s[:, :, 0, :] = g[0, 0] * s0 + g[0, 1] * s1
        s[:, :, 1, :] = g[1, 0] * s0 + g[1, 1] * s1
        return state

    def apply_2q(state, g4, q1, q2):
        g = g4.reshape(2, 2, 2, 2)
        if q1 > q2:
            g = np.transpose(g, (1, 0, 3, 2))
            q1, q2 = q2, q1
        A = 1 << q1
        M = 1 << (q2 - q1 - 1)
        s = state.reshape(DIM, A, 2, M, 2, -1)
        blocks = [s[:, :, c, :, d, :].copy() for c in (0, 1) for d in (0, 1)]
        for a in (0, 1):
            for b in (0, 1):
                acc = None
                for c in (0, 1):
                    for d in (0, 1):
                        coef = g[a, b, c, d]
                        if coef == 0:
                            continue
                        term = coef * blocks[2 * c + d]
                        acc = term if acc is None else acc + term
                s[:, :, a, :, b, :] = 0 if acc is None else acc
        return state

    def rot_matrix(phi, theta, omega):
        ct, st = np.cos(theta / 2), np.sin(theta / 2)
        return np.array(
            [[np.exp(-0.5j * (phi + omega)) * ct, -np.exp(0.5j * (phi - omega)) * st],
             [np.exp(-0.5j * (phi - omega)) * st, np.exp(0.5j * (phi + omega)) * ct]]
        )

    CNOT = np.array([[1, 0, 0, 0], [0, 1, 0, 0], [0, 0, 0, 1], [0, 0, 1, 0]], dtype)
    I4 = np.eye(4, dtype=dtype)
    XX = np.array([[0, 0, 0, 1], [0, 0, 1, 0], [0, 1, 0, 0], [1, 0, 0, 0]], dtype)
    YY = np.array([[0, 0, 0, -1], [0, 0, 1, 0], [0, 1, 0, 0], [-1, 0, 0, 0]], dtype)

    n_layers = w.shape[0]
    for l in range(n_layers):
        wl = w[l]
        for q in range(NQ):
            state = apply_1q(state, rot_matrix(*wl[q]), q)
        for q in range(NQ):
            state = apply_2q(state, CNOT, q, (q + 1) % NQ)
        c, s_ = np.cos(wl[0, 0] / 2), np.sin(wl[0, 0] / 2)
        state = apply_2q(state, c * I4 + (-1j * s_) * XX, 0, 1)
        c, s_ = np.cos(wl[0, 1] / 2), np.sin(wl[0, 1] / 2)
        state = apply_2q(state, c * I4 + (-1j * s_) * YY, 1, 2)
        e, ec = np.exp(-0.5j * wl[0, 2]), np.exp(0.5j * wl[0, 2])
        state = apply_2q(state, np.diag(np.array([e, ec, ec, e])), 2, 3)
    return state


def _host_prepare(x: np.ndarray, weights: np.ndarray):
    M = _build_circuit_matrix(weights)
    pc = np.array([bin(k).count("1") for k in range(DIM)])
    W = ((-1j) ** pc)[:, None] * M
    wr = W.real.astype(np.float16)   # (1024, 1024) [k, n]
    wi = W.imag.astype(np.float16)

    # wt[j, p, s, c]: j = out-amp chunk (8 total), p = in-amp within chunk,
    # s in 0..7 -> (in-chunk ko=s, real), 8..15 -> (ko=s-8, imag),
    # s = 16 -> Z-mask rows: wt[j, p, 16, q] = 1 - 2*bit_q(j*128 + p)
    wr4 = wr.reshape(KC, P, 8, P).transpose(2, 1, 0, 3)  # [j, p, ko, c]
    wi4 = wi.reshape(KC, P, 8, P).transpose(2, 1, 0, 3)
    wt = np.zeros((8, P, 17, P), dtype=np.float16)
    wt[:, :, 0:8, :] = wr4
    wt[:, :, 8:16, :] = wi4
    n = np.arange(DIM)
    zm = (1 - 2 * ((n[:, None] >> (NQ - 1 - np.arange(NQ))[None, :]) & 1)).astype(
        np.float16
    )  # (1024, 10)
    wt[:, :, 16, :NQ] = zm.reshape(8, P, NQ)
    # per-core W half: even core -> chunks 0:4, odd core -> 4:8
    wt_halves = [np.ascontiguousarray(wt[0:4]), np.ascontiguousarray(wt[4:8])]

    # full embedded state, transposed per sample block:
    # mt[p, k, b] = m[b, k*128+p]
    xd = x.astype(np.float64)
    c = np.cos(xd / 2)
    s = np.sin(xd / 2)
    B = x.shape[0]
    m = np.ones((B, 1))
    for q in range(NQ):
        f = np.stack([c[:, q], s[:, q]], axis=1)  # (B, 2)
        m = (m[:, :, None] * f[:, None, :]).reshape(B, -1)
    m = m.astype(np.float16)  # (B, 1024), amp bit order: qubit 0 = MSB
    mts = []
    for i in range(NPAIR):
        blk = m[i * BPC:(i + 1) * BPC]               # (512, 1024)
        mt = blk.T.reshape(KC, P, BPC).transpose(1, 0, 2)  # [p, k, b]
        mts.append(np.ascontiguousarray(mt))
    return mts, wt_halves


# ----------------------------------------------------------------------------
# Bass kernel (per-core program; SPMD across 8 cores)
# ----------------------------------------------------------------------------

def _build_bass() -> bass.Bass:
    nc = bacc.Bacc(trn_type="TRN2")

    mt_d = nc.dram_tensor("mt", (P, KC, BPC), F16, kind="ExternalInput")
    wt_d = nc.dram_tensor("wt", (JC, P, 17, P), F16, kind="ExternalInput")
    out_d = nc.dram_tensor("out", (NQ, BPC), F32, kind="ExternalOutput")

    H = BPC // 2  # column half for the last chunk's pipelined tail

    with TileContext(nc) as tc:
        with (
            tc.tile_pool(name="wpool", bufs=1) as w_pool,
            tc.tile_pool(name="work", bufs=1) as work_pool,
            tc.tile_pool(name="sq", bufs=3) as sq_pool,
            tc.tile_pool(name="mpsum", bufs=4, space="PSUM") as mpsum,
            tc.tile_pool(name="zpsum", bufs=1, space="PSUM") as zpsum,
            tc.tile_pool(name="wpsum", bufs=1, space="PSUM") as wpsum,
        ):
            # ---- DMA plan: two HWDGE queues, granules in consumption order.
            mt_sb = work_pool.tile([P, KC, BPC], F16, name="mt")
            w_sb = [w_pool.tile([P, 17, P], F16, name=f"w_{j}") for j in range(JC)]

            nc.sync.dma_start(mt_sb[:, 0:2, :], mt_d[:, 0:2, :])
            nc.scalar.dma_start(mt_sb[:, 2:4, :], mt_d[:, 2:4, :])
            nc.sync.dma_start(w_sb[0][:, 0:4, :], wt_d[0, :, 0:4, :])
            nc.scalar.dma_start(w_sb[0][:, 8:12, :], wt_d[0, :, 8:12, :])
            nc.sync.dma_start(mt_sb[:, 4:6, :], mt_d[:, 4:6, :])
            nc.scalar.dma_start(mt_sb[:, 6:8, :], mt_d[:, 6:8, :])
            nc.sync.dma_start(w_sb[0][:, 4:8, :], wt_d[0, :, 4:8, :])
            nc.scalar.dma_start(w_sb[0][:, 12:17, :], wt_d[0, :, 12:17, :])
            for j in range(1, JC):
                nc.sync.dma_start(w_sb[j][:, 0:8, :], wt_d[j, :, 0:8, :])
                nc.scalar.dma_start(w_sb[j][:, 8:17, :], wt_d[j, :, 8:17, :])

            # ---- PE warm-up on a memset dummy: ramps the tensor-engine clock
            # during the initial DMA wait.
            dummy = work_pool.tile([P, BPC], F16, name="dummy")
            nc.vector.memset(dummy, 0.0)
            ps_w = wpsum.tile([P, BPC], F32, name="psw")
            for _ in range(20):
                nc.tensor.matmul(ps_w, lhsT=dummy[:, 0:P], rhs=dummy,
                                 start=True, stop=True)

            # ---- main stream.
            zp = zpsum.tile([32, BPC], F32, name="zp")
            zout = work_pool.tile([NQ, BPC], F32, name="zout")
            pending_z = []  # (p_plane, chunk) awaiting their Z matmul

            def issue_z(stop):
                for p_t, jj in pending_z:
                    nc.tensor.matmul(
                        zp, lhsT=w_sb[jj][:, 16, 0:32], rhs=p_t,
                        start=(jj == 0), stop=stop,
                        skip_group_check=True,
                    )
                pending_z.clear()

            for j in range(JC):
                last = j == JC - 1
                ps_r = mpsum.tile([P, BPC], F32, name="psr", tag="mmps")
                for k in range(KC):
                    nc.tensor.matmul(
                        ps_r, lhsT=w_sb[j][:, k, :], rhs=mt_sb[:, k, :],
                        start=(k == 0), stop=(k == KC - 1),
                    )
                # previous chunk's Z rides here, while ACT/DVE build p[j]
                issue_z(False)
                ps_i = mpsum.tile([P, BPC], F32, name="psi", tag="mmps")
                p_j = sq_pool.tile([P, BPC], F16, name="p")
                if last:
                    # column-split so the probs chain pipelines with the
                    # final im matmuls and only a short tail remains
                    for h in range(2):
                        lo, hi = h * H, (h + 1) * H
                        for k in range(KC):
                            nc.tensor.matmul(
                                ps_i[:, lo:hi], lhsT=w_sb[j][:, 8 + k, :],
                                rhs=mt_sb[:, k, lo:hi],
                                start=(k == 0), stop=(k == KC - 1),
                                skip_group_check=True,
                            )
                        sq_r = sq_pool.tile([P, H], F16, name="sqr")
                        sq_i = sq_pool.tile([P, H], F16, name="sqi")
                        nc.scalar.square(sq_r, ps_r[:, lo:hi])
                        nc.scalar.square(sq_i, ps_i[:, lo:hi])
                        nc.vector.tensor_tensor(p_j[:, lo:hi], sq_r, sq_i, ADD)
                else:
                    for k in range(KC):
                        nc.tensor.matmul(
                            ps_i, lhsT=w_sb[j][:, 8 + k, :], rhs=mt_sb[:, k, :],
                            start=(k == 0), stop=(k == KC - 1),
                        )
                    sq_r = sq_pool.tile([P, BPC], F16, name="sqr")
                    sq_i = sq_pool.tile([P, BPC], F16, name="sqi")
                    nc.scalar.square(sq_r, ps_r)
                    nc.scalar.square(sq_i, ps_i)
                    nc.vector.tensor_tensor(p_j, sq_r, sq_i, ADD)
                pending_z.append((p_j, j))
            issue_z(True)

            nc.vector.tensor_copy(zout[:], zp[0:NQ, :])
            nc.sync.dma_start(out_d[:], zout[:])

    nc.finalize()
    return nc


# ----------------------------------------------------------------------------
# Entry point
# ----------------------------------------------------------------------------

def kernel(x: np.ndarray, weights: np.ndarray, _trace: bool = False) -> np.ndarray:
    global LAST_RESULT
    x = np.asarray(x, dtype=np.float32)
    weights = np.asarray(weights, dtype=np.float32)

    mts, wt_halves = _host_prepare(x, weights)

    nc = _build_bass()
    in_maps = [
        {"mt": mts[i // 2], "wt": wt_halves[i % 2]} for i in range(NCORES)
    ]
    res = run_bass_kernel_spmd(
        nc, in_maps, core_ids=list(range(NCORES)), trace=_trace
    )
    LAST_RESULT = res
    blocks = []
    for c in range(NPAIR):
        z = np.asarray(res.results[2 * c]["out"]) + np.asarray(
            res.results[2 * c + 1]["out"])
        blocks.append(z.T)  # (512, 10)
    out = np.concatenate(blocks, axis=0)
    return np.ascontiguousarray(out).astype(np.float32)


# revision 13
# speedup vs baseline: 1.1098x; 1.1098x over previous
"""Trainium2 Bass kernel for nn_ComplexQuantumLayer (10-qubit circuit, batch 2048).

Math: the circuit after the RX AngleEmbedding is a fixed unitary U (depends only
on `weights`), and the embedded state is a Kronecker product
  psi0[b] = (-i)^popcount(j) * m[b, j],   m[b] = kron_q [cos(x_bq/2), sin(x_bq/2)].
Folding the phase into W = diag(phase) @ U^T gives  psi = m @ W  with m REAL.
Per sample the device does two real (1024,1024) matvecs (fp16 operands, fp32
PSUM), |psi|^2 via ACT/DVE squares, and the ten PauliZ sums as small matmuls
against a +/-1 mask matrix (lhsT zero-padded to 32 cols, full rate).

v5 design — 2D sharding (batch x out-amp), two column passes:
  Cores are paired: pair c covers samples [512c, 512c+512); the even core
  computes out-amp chunks 0-3, the odd core chunks 4-7. Each core loads HALF
  of W (2.23MB) plus mt for its sample block (1MB) = 3.25MB, well under the
  ~330GB/s per-core DMA ceiling for the ~16us PE stream. The host adds the
  two partial Z sums of a pair.

  - The host sends m TRANSPOSED (amp-major): mt[p, k, b] = m[b, k*128+p].
    No on-device Kronecker tree or transposes.
  - The 512 samples are processed as two independent 256-column passes
    (v2-proven shapes: [128,256] matmuls, half-bank PSUM tiles). Pass 0
    needs only mt cols 0:256 + W; its Z output flushes to DRAM while pass 1
    runs. All W tiles are resident for pass 1, so its stream has no DMA
    dependencies at all.
  - Probs: ACT squares ps_r; DVE copies ps_i to fp32 and squares to fp16.
    Re and im prob planes feed SEPARATE Z-mask matmuls (16 per pass worth
    ~0.9us PE) — no cross-engine add, so the chain after the last im matmul
    is short. Z matmuls trail their chunk by one re-pass.
  - DMA rings alternate across the two HWDGE queues in consumption order:
      sync  : mt0[k0:4] | W0re[0:4] | W0re[4:8] | W1re | W2re | W3re | mt1[k0:4]
      scalar: mt0[k4:8] | W0im[8:12] | W0im[12:17] | W1im | W2im | W3im | mt1[k4:8]
  - 12 PE warm-up matmuls on a memset dummy ride the initial DMA wait
    (the tensor clock ramps over the first ~13us of the kernel).
"""

import numpy as np

import concourse.bass as bass
import concourse.bacc as bacc
import concourse.mybir as mybir
from concourse.bass_utils import run_bass_kernel_spmd
from concourse.tile import TileContext

NQ = 10
DIM = 1 << NQ          # 1024
BATCH = 2048
NCORES = 8
NPAIR = NCORES // 2    # 4 sample blocks
BPC = BATCH // NPAIR   # 512 samples per core (shared by a pair)
HB = BPC // 2          # 256 samples per column pass
P = 128                # partitions
KC = DIM // P          # in-amp chunks = 8
JC = 4                 # out-amp chunks per core (half of 8)

F32 = mybir.dt.float32
F16 = mybir.dt.float16
MUL = mybir.AluOpType.mult
ADD = mybir.AluOpType.add

LAST_RESULT = None  # BassKernelResults of the most recent run (for test harness)


# ----------------------------------------------------------------------------
# Host-side preprocessing: circuit unitary from weights (numpy, ~2s)
# ----------------------------------------------------------------------------

def _build_circuit_matrix(weights: np.ndarray, dtype=np.complex128) -> np.ndarray:
    """M = U^T: the reference circuit (post-embedding) applied to identity rows."""
    w = weights.astype(np.float64)
    state = np.eye(DIM, dtype=dtype)

    def apply_1q(state, g, q):
        s = state.reshape(DIM, 1 << q, 2, -1)
        s0 = s[:, :, 0, :].copy()
        s1 = s[:, :, 1, :].copy()
        s[:, :, 0, :] = g[0, 0] * s0 + g[0, 1] * s1
        s[:, :, 1, :] = g[1, 0] * s0 + g[1, 1] * s1
        return state

    def apply_2q(state, g4, q1, q2):
        g = g4.reshape(2, 2, 2, 2)
        if q1 > q2:
            g = np.transpose(g, (1, 0, 3, 2))
            q1, q2 = q2, q1
        A = 1 << q1
        M = 1 << (q2 - q1 - 1)
        s = state.reshape(DIM, A, 2, M, 2, -1)
        blocks = [s[:, :, c, :, d, :].copy() for c in (0, 1) for d in (0, 1)]
        for a in (0, 1):
            for b in (0, 1):
                acc = None
                for c in (0, 1):
                    for d in (0, 1):
                        coef = g[a, b, c, d]
                        if coef == 0:
                            continue
                        term = coef * blocks[2 * c + d]
                        acc = term if acc is None else acc + term
                s[:, :, a, :, b, :] = 0 if acc is None else acc
        return state

    def rot_matrix(phi, theta, omega):
        ct, st = np.cos(theta / 2), np.sin(theta / 2)
        return np.array(
            [[np.exp(-0.5j * (phi + omega)) * ct, -np.exp(0.5j * (phi - omega)) * st],
             [np.exp(-0.5j * (phi - omega)) * st, np.exp(0.5j * (phi + omega)) * ct]]
        )

    CNOT = np.array([[1, 0, 0, 0], [0, 1, 0, 0], [0, 0, 0, 1], [0, 0, 1, 0]], dtype)
    I4 = np.eye(4, dtype=dtype)
    XX = np.array([[0, 0, 0, 1], [0, 0, 1, 0], [0, 1, 0, 0], [1, 0, 0, 0]], dtype)
    YY = np.array([[0, 0, 0, -1], [0, 0, 1, 0], [0, 1, 0, 0], [-1, 0, 0, 0]], dtype)

    n_layers = w.shape[0]
    for l in range(n_layers):
        wl = w[l]
        for q in range(NQ):
            state = apply_1q(state, rot_matrix(*wl[q]), q)
        for q in range(NQ):
            state = apply_2q(state, CNOT, q, (q + 1) % NQ)
        c, s_ = np.cos(wl[0, 0] / 2), np.sin(wl[0, 0] / 2)
        state = apply_2q(state, c * I4 + (-1j * s_) * XX, 0, 1)
        c, s_ = np.cos(wl[0, 1] / 2), np.sin(wl[0, 1] / 2)
        state = apply_2q(state, c * I4 + (-1j * s_) * YY, 1, 2)
        e, ec = np.exp(-0.5j * wl[0, 2]), np.exp(0.5j * wl[0, 2])
        state = apply_2q(state, np.diag(np.array([e, ec, ec, e])), 2, 3)
    return state


def _host_prepare(x: np.ndarray, weights: np.ndarray):
    M = _build_circuit_matrix(weights)
    pc = np.array([bin(k).count("1") for k in range(DIM)])
    W = ((-1j) ** pc)[:, None] * M
    wr = W.real.astype(np.float16)   # (1024, 1024) [k, n]
    wi = W.imag.astype(np.float16)

    # wt[j, p, s, c]: j = out-amp chunk (8 total), p = in-amp within chunk,
    # s in 0..7 -> (in-chunk ko=s, real), 8..15 -> (ko=s-8, imag),
    # s = 16 -> Z-mask rows: wt[j, p, 16, q] = 1 - 2*bit_q(j*128 + p)
    wr4 = wr.reshape(KC, P, 8, P).transpose(2, 1, 0, 3)  # [j, p, ko, c]
    wi4 = wi.reshape(KC, P, 8, P).transpose(2, 1, 0, 3)
    wt = np.zeros((8, P, 17, P), dtype=np.float16)
    wt[:, :, 0:8, :] = wr4
    wt[:, :, 8:16, :] = wi4
    n = np.arange(DIM)
    zm = (1 - 2 * ((n[:, None] >> (NQ - 1 - np.arange(NQ))[None, :]) & 1)).astype(
        np.float16
    )  # (1024, 10)
    wt[:, :, 16, :NQ] = zm.reshape(8, P, NQ)
    # per-core W half: even core -> chunks 0:4, odd core -> 4:8
    wt_halves = [np.ascontiguousarray(wt[0:4]), np.ascontiguousarray(wt[4:8])]

    # full embedded state, transposed per sample block:
    # mt[p, k, b] = m[b, k*128+p]
    xd = x.astype(np.float64)
    c = np.cos(xd / 2)
    s = np.sin(xd / 2)
    B = x.shape[0]
    m = np.ones((B, 1))
    for q in range(NQ):
        f = np.stack([c[:, q], s[:, q]], axis=1)  # (B, 2)
        m = (m[:, :, None] * f[:, None, :]).reshape(B, -1)
    m = m.astype(np.float16)  # (B, 1024), amp bit order: qubit 0 = MSB
    mts = []
    for i in range(NPAIR):
        blk = m[i * BPC:(i + 1) * BPC]               # (512, 1024)
        mt = blk.T.reshape(KC, P, BPC).transpose(1, 0, 2)  # [p, k, b]
        mts.append(np.ascontiguousarray(mt))
    return mts, wt_halves


# ----------------------------------------------------------------------------
# Bass kernel (per-core program; SPMD across 8 cores)
# ----------------------------------------------------------------------------

def _build_bass() -> bass.Bass:
    nc = bacc.Bacc(trn_type="TRN2")

    mt_d = nc.dram_tensor("mt", (P, KC, BPC), F16, kind="ExternalInput")
    wt_d = nc.dram_tensor("wt", (JC, P, 17, P), F16, kind="ExternalInput")
    out_d = nc.dram_tensor("out", (2, NQ, HB), F32, kind="ExternalOutput")

    with TileContext(nc) as tc:
        with (
            tc.tile_pool(name="wpool", bufs=1) as w_pool,
            tc.tile_pool(name="work", bufs=1) as work_pool,
            tc.tile_pool(name="sq", bufs=3) as sq_pool,
            tc.tile_pool(name="mpsum", bufs=3, space="PSUM") as mpsum,
            tc.tile_pool(name="zpsum", bufs=2, space="PSUM") as zpsum,
            tc.tile_pool(name="wpsum", bufs=1, space="PSUM") as wpsum,
        ):
            # ---- DMA plan: two HWDGE queues, granules in consumption order.
            mt0 = work_pool.tile([P, KC, HB], F16, name="mt0")
            mt1 = work_pool.tile([P, KC, HB], F16, name="mt1")
            w_sb = [w_pool.tile([P, 17, P], F16, name=f"w_{j}") for j in range(JC)]

            nc.sync.dma_start(mt0[:, 0:4, :], mt_d[:, 0:4, 0:HB])
            nc.scalar.dma_start(mt0[:, 4:8, :], mt_d[:, 4:8, 0:HB])
            nc.sync.dma_start(w_sb[0][:, 0:4, :], wt_d[0, :, 0:4, :])
            nc.scalar.dma_start(w_sb[0][:, 8:12, :], wt_d[0, :, 8:12, :])
            nc.sync.dma_start(w_sb[0][:, 4:8, :], wt_d[0, :, 4:8, :])
            nc.scalar.dma_start(w_sb[0][:, 12:17, :], wt_d[0, :, 12:17, :])
            for j in range(1, JC):
                nc.sync.dma_start(w_sb[j][:, 0:8, :], wt_d[j, :, 0:8, :])
                nc.scalar.dma_start(w_sb[j][:, 8:17, :], wt_d[j, :, 8:17, :])
            nc.sync.dma_start(mt1[:, 0:4, :], mt_d[:, 0:4, HB:BPC])
            nc.scalar.dma_start(mt1[:, 4:8, :], mt_d[:, 4:8, HB:BPC])

            # ---- PE warm-up on a memset dummy (rides the DMA wait).
            dummy = work_pool.tile([P, HB], F16, name="dummy")
            nc.gpsimd.memset(dummy, 0.0)
            ps_w = wpsum.tile([P, HB], F32, name="psw")
            for _ in range(12):
                nc.tensor.matmul(ps_w, lhsT=dummy[:, 0:P], rhs=dummy,
                                 start=True, stop=True)

            # ---- two independent column passes.
            def col_pass(h, mt_sb):
                zp = zpsum.tile([32, HB], F32, name=f"zp{h}")
                zout = work_pool.tile([NQ, HB], F32, name=f"zout{h}")
                pend = []      # (prob_plane, chunk) awaiting Z
                z_state = [0]  # number of Z matmuls issued into zp

                def issue_z():
                    for plane, jj in pend:
                        nc.tensor.matmul(
                            zp, lhsT=w_sb[jj][:, 16, 0:32], rhs=plane,
                            start=(z_state[0] == 0),
                            stop=(z_state[0] == 2 * JC - 1),
                            skip_group_check=True,
                        )
                        z_state[0] += 1
                    pend.clear()

                for j in range(JC):
                    ps_r = mpsum.tile([P, HB], F32, name="psr", tag="mmps")
                    for k in range(KC):
                        nc.tensor.matmul(
                            ps_r, lhsT=w_sb[j][:, k, :], rhs=mt_sb[:, k, :],
                            start=(k == 0), stop=(k == KC - 1),
                        )
                    issue_z()  # previous chunk's re/im Z ride here
                    ps_i = mpsum.tile([P, HB], F32, name="psi", tag="mmps")
                    for k in range(KC):
                        nc.tensor.matmul(
                            ps_i, lhsT=w_sb[j][:, 8 + k, :], rhs=mt_sb[:, k, :],
                            start=(k == 0), stop=(k == KC - 1),
                        )
                    sq_r = sq_pool.tile([P, HB], F16, name="sqr")
                    nc.scalar.square(sq_r, ps_r)
                    ci = sq_pool.tile([P, HB], F32, name="ci")
                    sq_i = sq_pool.tile([P, HB], F16, name="sqi")
                    nc.vector.tensor_copy(ci, ps_i)
                    nc.vector.tensor_tensor(sq_i, ci, ci, MUL)
                    pend.append((sq_r, j))
                    pend.append((sq_i, j))
                issue_z()  # last chunk's Z
                nc.scalar.copy(zout[:], zp[0:NQ, :])
                nc.sync.dma_start(out_d[h], zout[:])

            col_pass(0, mt0)
            col_pass(1, mt1)

    nc.finalize()
    return nc


# ----------------------------------------------------------------------------
# Entry point
# ----------------------------------------------------------------------------

def kernel(x: np.ndarray, weights: np.ndarray, _trace: bool = False) -> np.ndarray:
    global LAST_RESULT
    x = np.asarray(x, dtype=np.float32)
    weights = np.asarray(weights, dtype=np.float32)

    mts, wt_halves = _host_prepare(x, weights)

    nc = _build_bass()
    in_maps = [
        {"mt": mts[i // 2], "wt": wt_halves[i % 2]} for i in range(NCORES)
    ]
    res = run_bass_kernel_spmd(
        nc, in_maps, core_ids=list(range(NCORES)), trace=_trace
    )
    LAST_RESULT = res
    blocks = []
    for c in range(NPAIR):
        z = np.asarray(res.results[2 * c]["out"]) + np.asarray(
            res.results[2 * c + 1]["out"])          # (2, NQ, 256)
        blocks.append(np.concatenate([z[0], z[1]], axis=-1).T)  # (512, 10)
    out = np.concatenate(blocks, axis=0)
    return np.ascontiguousarray(out).astype(np.float32)
